# revision 13
# baseline (speedup 1.0000x reference)
"""CP-decomposed embedding lookup kernel for Trainium2 (8 NeuronCores).

Math (matches the CPEmbedding reference):
    A = khatri_rao(U0, U1, U2)            # [500000, 32]
    B = khatri_rao(V0, V1)                # [128, 32]
    out = (A @ B.T)[x]                    # [1024, 200, 128]

We never materialize A. Per lookup x = a*5000 + b*50 + c = j*50 + c:
    w[r]   = U01[j, r] * U2[c, r]         where U01[j=a*100+b, r] = U0[a,r]*U1[b,r]
    out[x] = w @ B.T

Sharding: the CP factors are tiny and replicated; the 204800 lookups are
sharded evenly across the 8 cores (data parallel over lookups), so each core
computes a contiguous [25600, 128] output slice and the host concatenates.

Within a core, lookup l = p*200 + t is assigned to SBUF partition p, tile
slot t.  This makes each partition's 200 output rows contiguous in DRAM, so
the output DMA runs at multi-KB descriptors instead of one 512B descriptor
per row (the SDMA path is descriptor-rate limited, not BW limited).

Device pipeline per core:
  1. one-time: build the U01 table [10000 rows, 64-f32-padded] in DRAM via a
     single broadcast DVE multiply, build a padded U2 table [50, 64], and
     build B^T [32, 128] (bf16) replicated at 4 partition blocks.
  2. per chunk of 1536 lookups: two batched SWDGE dma_gathers (row per
     lookup) round-robined across all 4 SWDGE queues (each queue has its own
     Q7 cpu pair generating descriptors; a single queue is desc-gen bound),
     DVE multiply -> W [128, 32] bf16 per 128-lookup tile, PE transpose of 4
     tiles at once (bf16 matmul vs identity) -> W^T stationaries at row
     groups 0/32/64/96, 4 bf16 matmuls against B^T -> psum, ACT/DVE copy to
     a chunk-wide SBUF tile, one contiguous-per-partition HWDGE DMA per
     chunk to the output slice.
"""

import numpy as np

import concourse.bacc as bacc
import concourse.bass as bass
import concourse.mybir as mybir
import concourse.tile as tile
from concourse import bass_utils
from concourse.ap import AP
from concourse.masks import make_identity

# Problem constants (hardcoded per the harness contract).
VOC = (100, 100, 50)  # a, b, c
EMB = (8, 16)  # d, e'
RANK = 32
E = EMB[0] * EMB[1]  # 128
N_CORES = 8
X_SHAPE = (1024, 200)
N_TOTAL = X_SHAPE[0] * X_SHAPE[1]  # 204800
P = 128

U01_ROWS = VOC[0] * VOC[1]  # 10000
ROW_PAD = 64  # table row = 64 f32 = 256 B (dma_gather elem_size constraint)


class Cfg:
    def __init__(self, n_core, chunks):
        assert n_core % P == 0
        self.n_core = n_core
        self.n_tiles = n_core // P
        self.chunks = list(chunks)  # tiles per chunk; each % 4 == 0
        assert sum(self.chunks) == self.n_tiles
        assert all(c % 4 == 0 for c in self.chunks)
        self.max_chunk = max(self.chunks)
        self.idx_cols = n_core // 16  # wrapped idx layout [16 -> 128, idx_cols]


# 25600 lookups; small chunks bound the pipeline-fill latency and keep the
# gather/compute pipeline fine-grained across the 4 SWDGE queues.
FULL_CFG = Cfg(N_TOTAL // N_CORES, [12] * 16 + [8])

F32 = mybir.dt.float32
BF16 = mybir.dt.bfloat16
I16 = mybir.dt.int16

N_QUEUES = 4


def dma_gather_narrow(eng, out_ap, in_ap, idxs_ap, num_idxs, elem_size, elem_step,
                      queue_num=0):
    """dma_gather for the HBM-source non-transpose case with elem_size_bytes
    not necessarily a multiple of 256 (the Q7 ucode loops over bytes; only the
    row STRIDE must stay 256B-aligned). Mirrors bass.dma_gather's lowering.
    HW-validated: gathering 128B rows from a 256B-strided table is exact."""
    nc = eng.bass
    stride_bytes = elem_step * mybir.dt.size(in_ap.dtype)
    assert stride_bytes % 256 == 0 and stride_bytes // 256 < 256
    assert idxs_ap.dtype == I16
    _in_ap = eng.lower_ap_dma(in_ap, for_custom_bir_dma=True)
    _idxs_ap = eng.lower_ap(idxs_ap)
    _out_ap = eng.lower_ap(out_ap)
    return eng.add_instruction(
        mybir.InstDMAGatherAnt(
            name=nc.get_next_instruction_name(),
            ins=[*_in_ap, _idxs_ap, eng.lower_val_access(eng.to_reg(num_idxs))],
            outs=[_out_ap],
            transpose=False,
            num_idxs=num_idxs,
            elem_size=elem_size,
            stride_bytes_256=stride_bytes // 256,
            gen_mode=0,
            single_packet=False,
            queue_num=queue_num,
            sbuf_tokens_per_rank=0,
            sbuf_free_dim_per_rank=0,
            sbuf_free_dim_pad_per_rank=0,
            sbuf_byte_offset=0,
        )
    )


def build_program(cfg: Cfg, mode: str = "full"):
    """Build the SPMD single-core program; per-core differences are inputs.

    mode: "full" | "nogather" (memset the gather buffers instead of SWDGE
    gathers; for HW bisection only)."""
    nc = bacc.Bacc("TRN2", target_bir_lowering=False, debug=False,
                   num_swdge_queues=N_QUEUES)

    # ---- DRAM I/O ----
    jidx_d = nc.dram_tensor("jidx", [P, cfg.idx_cols], I16, kind="ExternalInput")
    cidx_d = nc.dram_tensor("cidx", [P, cfg.idx_cols], I16, kind="ExternalInput")
    u0_d = nc.dram_tensor("u0", [VOC[0], RANK], F32, kind="ExternalInput")
    u1rep_d = nc.dram_tensor("u1rep", [VOC[0], VOC[1] * RANK], F32, kind="ExternalInput")
    u2_d = nc.dram_tensor("u2", [VOC[2], RANK], F32, kind="ExternalInput")
    v0t4_d = nc.dram_tensor("v0t4", [P, EMB[0]], F32, kind="ExternalInput")
    v1t4_d = nc.dram_tensor("v1t4", [P, EMB[1]], F32, kind="ExternalInput")
    out_d = nc.dram_tensor("out", [cfg.n_core, E], F32, kind="ExternalOutput")
    # DRAM rows ordered so partition p's tiles are contiguous: row p*T + t.
    out_view = out_d.ap().rearrange("(p t) e -> p t e", p=P)

    with tile.TileContext(nc) as tc:
        const = tc.alloc_tile_pool(name="const", bufs=1)
        dram = tc.alloc_tile_pool(name="dram", bufs=1, space="DRAM")

        # ---------- one-time setup ----------
        # U01 table build heads the critical chain that gates the first real
        # gather: u1rep DMA -> DVE mul -> table DMA -> gather.  The mul and
        # the table DMA are split into four b-waves so they pipeline.
        # u01s[a, b, r] = U0[a, r] * U1[b, r]
        u0s = const.tile([VOC[0], RANK], F32)
        u1s = const.tile([VOC[0], VOC[1] * RANK], F32)
        nc.sync.dma_start(u0s[:], u0_d.ap())
        nc.sync.dma_start(u1s[:], u1rep_d.ap())
        u01s = const.tile([VOC[0], VOC[1] * RANK], F32)
        u01_tab = dram.tile([U01_ROWS, ROW_PAD], F32)
        NB = 4
        BH = VOC[1] // NB
        for h in range(NB):
            bs = slice(h * BH, (h + 1) * BH)
            nc.vector.tensor_tensor(
                out=u01s[:].rearrange("p (b r) -> p b r", r=RANK)[:, bs, :],
                in0=u0s[:][:, None, :].to_broadcast([VOC[0], BH, RANK]),
                in1=u1s[:].rearrange("p (b r) -> p b r", r=RANK)[:, bs, :],
                op=mybir.AluOpType.mult,
            )
            nc.sync.dma_start(
                u01_tab[:][:, 0:RANK].rearrange("(a b) r -> a b r", a=VOC[0])[
                    :, bs, :
                ],
                u01s[:].rearrange("p (b r) -> p b r", r=RANK)[:, bs, :],
            )

        # U2 table (tiny, separate tensor) so its gathers' descriptor
        # generation is not gated on the U01 table build.
        u2s = const.tile([VOC[2], RANK], F32)
        nc.scalar.dma_start(u2s[:], u2_d.ap())
        u2_tab = dram.tile([VOC[2], ROW_PAD], F32)
        nc.scalar.dma_start(u2_tab[:][:, 0:RANK], u2s[:])

        # idx tiles (gate only the Pool-engine descriptor generation)
        jidx = const.tile([P, cfg.idx_cols], I16)
        cidx = const.tile([P, cfg.idx_cols], I16)
        nc.scalar.dma_start(jidx[:], jidx_d.ap())
        nc.scalar.dma_start(cidx[:], cidx_d.ap())

        ident = const.tile([P, P], BF16)
        make_identity(nc, ident[:])

        # B^T (bf16) replicated at the 4 partition blocks:
        # bt[32g + r, d*16+e'] = V0[d, r] * V1[e', r]
        v0s = const.tile([P, EMB[0]], F32)
        v1s = const.tile([P, EMB[1]], F32)
        nc.scalar.dma_start(v0s[:], v0t4_d.ap())
        nc.scalar.dma_start(v1s[:], v1t4_d.ap())
        bt = const.tile([P, E], BF16)
        nc.vector.tensor_tensor(
            out=bt[:].rearrange("p (d e) -> p d e", e=EMB[1]),
            in0=v0s[:][:, :, None].to_broadcast([P, EMB[0], EMB[1]]),
            in1=v1s[:][:, None, :].to_broadcast([P, EMB[0], EMB[1]]),
            op=mybir.AluOpType.mult,
        )

        # ---------- main pipeline ----------
        g1p = tc.alloc_tile_pool(name="g1", bufs=3)
        g2p = tc.alloc_tile_pool(name="g2", bufs=3)
        wp = tc.alloc_tile_pool(name="w", bufs=2)
        wtpp = tc.alloc_tile_pool(name="wtp", bufs=2, space="PSUM")
        wtsp = tc.alloc_tile_pool(name="wts", bufs=4)
        opp = tc.alloc_tile_pool(name="op", bufs=6, space="PSUM")
        osp = tc.alloc_tile_pool(name="os", bufs=2)

        tile0 = 0
        for ch, ctiles in enumerate(cfg.chunks):
            cidx0 = tile0 * P // 16
            icols = ctiles * P // 16
            g1 = g1p.tile([P, ctiles, RANK], F32, tag="g1")
            g2 = g2p.tile([P, ctiles, RANK], F32, tag="g2")
            if mode == "nogather":
                nc.gpsimd.memset(g1[:], 1.0)
                nc.gpsimd.memset(g2[:], 1.0)
            else:
                # Round-robin both gathers of a chunk onto different SWDGE
                # queues; each queue has a dedicated Q7 cpu pair, so 4 queues
                # give 4x the descriptor generation rate.
                dma_gather_narrow(
                    nc.gpsimd, g2[:], u2_tab[:][:, 0:RANK],
                    cidx[:][:, cidx0 : cidx0 + icols],
                    ctiles * P, RANK, ROW_PAD,
                    queue_num=(2 * ch) % N_QUEUES,
                )
                dma_gather_narrow(
                    nc.gpsimd, g1[:], u01_tab[:][:, 0:RANK],
                    jidx[:][:, cidx0 : cidx0 + icols],
                    ctiles * P, RANK, ROW_PAD,
                    queue_num=(2 * ch + 1) % N_QUEUES,
                )
            w = wp.tile([P, cfg.max_chunk * RANK], BF16, tag="w")
            out_sb = osp.tile([P, cfg.max_chunk * E], F32, tag="os")
            for pk in range(ctiles // 4):
                # per-pack W multiply: finer grain lets the first transpose
                # start as soon as the gathers land
                nc.vector.tensor_tensor(
                    out=w[:].rearrange("p (t r) -> p t r", r=RANK)[
                        :, pk * 4 : (pk + 1) * 4, :
                    ],
                    in0=g1[:][:, pk * 4 : (pk + 1) * 4, :],
                    in1=g2[:][:, pk * 4 : (pk + 1) * 4, :],
                    op=mybir.AluOpType.mult,
                )
                # W^T via plain matmul against identity (fp32 is_transpose
                # crashes the exec unit on this stack; W.T @ I is exact).
                wt_ps = wtpp.tile([P, P], F32)
                nc.tensor.matmul(
                    out=wt_ps[:],
                    lhsT=w[:][:, pk * P : (pk + 1) * P],
                    rhs=ident[:],
                    start=True,
                    stop=True,
                )
                wt = wtsp.tile([P, P], BF16)
                nc.vector.tensor_copy(wt[:], wt_ps[:])
                for g in range(4):
                    out_ps = opp.tile([P, E], F32, tag="ops")
                    nc.tensor.matmul(
                        out=out_ps[:],
                        lhsT=wt[:][g * RANK : (g + 1) * RANK, :],
                        rhs=bt[:][g * RANK : (g + 1) * RANK, :],
                        start=True,
                        stop=True,
                        tile_position=(g * RANK, 0),
                    )
                    dst = out_sb[:][:, (pk * 4 + g) * E : (pk * 4 + g + 1) * E]
                    if g % 2 == 0:
                        nc.scalar.copy(dst, out_ps[:])
                    else:
                        nc.vector.tensor_copy(dst, out_ps[:])
            # One DMA per chunk: partition p's ctiles rows are contiguous in
            # DRAM (rows p*T + tile0 .. + ctiles), so each partition is one
            # multi-KB descriptor run instead of per-row 512B descriptors.
            nc.sync.dma_start(
                out_view[:, tile0 : tile0 + ctiles, :],
                out_sb[:][:, 0 : ctiles * E].rearrange("p (t e) -> p t e", e=E),
            )
            tile0 += ctiles

        for pool in (osp, opp, wtsp, wtpp, wp, g2p, g1p, dram, const):
            pool.release()

    nc.compile()
    return nc


def wrap_idx(v: np.ndarray) -> np.ndarray:
    """Host-side routing prep: dma_gather wants index i at [i % 16, i // 16],
    replicated down all 128 partitions (8 Q7 cores x 16 partitions)."""
    w = v.astype(np.int16).reshape(-1, 16).T  # [16, cols]
    return np.ascontiguousarray(np.tile(w, (8, 1)))  # [128, cols]


_CACHE: dict = {}


def _get_program(cfg: Cfg):
    key = (cfg.n_core, tuple(cfg.chunks))
    if key not in _CACHE:
        _CACHE[key] = build_program(cfg)
    return _CACHE[key]


def make_in_maps(x, U0, U1, U2, V0, V1, cfg: Cfg, n_cores: int):
    xf = np.asarray(x).reshape(-1).astype(np.int64)
    j = (xf // VOC[2]).astype(np.int16)  # [0, 10000)
    c = (xf % VOC[2]).astype(np.int16)  # [0, 50)

    u0 = np.ascontiguousarray(np.asarray(U0, dtype=np.float32))
    u1rep = np.ascontiguousarray(
        np.broadcast_to(
            np.asarray(U1, dtype=np.float32).reshape(1, VOC[1] * RANK),
            (VOC[0], VOC[1] * RANK),
        )
    )
    u2 = np.ascontiguousarray(np.asarray(U2, dtype=np.float32))
    v0t4 = np.ascontiguousarray(np.tile(np.asarray(V0, dtype=np.float32).T, (4, 1)))
    v1t4 = np.ascontiguousarray(np.tile(np.asarray(V1, dtype=np.float32).T, (4, 1)))

    T = cfg.n_tiles
    in_maps = []
    for k in range(n_cores):
        sl = slice(k * cfg.n_core, (k + 1) * cfg.n_core)
        # Gather-row g serves (slot t=g//128, partition p=g%128) = lookup
        # p*T + t, so permute the per-core index vector accordingly.
        jg = j[sl].reshape(P, T).T.reshape(-1)
        cg = c[sl].reshape(P, T).T.reshape(-1)
        in_maps.append(
            {
                "jidx": wrap_idx(jg),
                "cidx": wrap_idx(cg),
                "u0": u0,
                "u1rep": u1rep,
                "u2": u2,
                "v0t4": v0t4,
                "v1t4": v1t4,
            }
        )
    return in_maps


def kernel(x, U0, U1, U2, V0, V1, _trace=False):
    cfg = FULL_CFG
    nc = _get_program(cfg)
    in_maps = make_in_maps(x, U0, U1, U2, V0, V1, cfg, N_CORES)
    res = bass_utils.run_bass_kernel_spmd(
        nc, in_maps, core_ids=list(range(N_CORES)), trace=_trace
    )
    outs = []
    for k in range(N_CORES):
        # Device rows are ordered (p, t) = lookup p*T + t: already the
        # natural per-core order.
        outs.append(res.results[k]["out"])
    out = np.concatenate(outs, axis=0)
    out = out.reshape(*np.asarray(x).shape, E).astype(np.float32)
    if _trace:
        kernel._last_result = res
    return out


# revision 15
# speedup vs baseline: 1.0195x; 1.0195x over previous
"""CP-decomposed embedding lookup kernel for Trainium2 (8 NeuronCores).

Math (matches the CPEmbedding reference):
    A = khatri_rao(U0, U1, U2)            # [500000, 32]
    B = khatri_rao(V0, V1)                # [128, 32]
    out = (A @ B.T)[x]                    # [1024, 200, 128]

We never materialize A. Per lookup x = a*5000 + b*50 + c = j*50 + c:
    w[r]   = U01[j, r] * U2[c, r]         where U01[j=a*100+b, r] = U0[a,r]*U1[b,r]
    out[x] = w @ B.T

Sharding: the CP factors are tiny and replicated; the 204800 lookups are
sharded evenly across the 8 cores (data parallel over lookups), so each core
computes a contiguous [25600, 128] output slice and the host concatenates.

Within a core, lookup l = p*200 + t is assigned to SBUF partition p, tile
slot t.  This makes each partition's 200 output rows contiguous in DRAM, so
the output DMA runs at multi-KB descriptors instead of one 512B descriptor
per row (the SDMA path is descriptor-rate limited, not BW limited).

Device pipeline per core:
  1. one-time: build the U01 table [10000 rows, 64-f32-padded] in DRAM via a
     single broadcast DVE multiply, build a padded U2 table [50, 64], and
     build B^T [32, 128] (bf16) replicated at 4 partition blocks.
  2. per chunk of 1536 lookups: two batched SWDGE dma_gathers (row per
     lookup) round-robined across all 4 SWDGE queues (each queue has its own
     Q7 cpu pair generating descriptors; a single queue is desc-gen bound),
     DVE multiply -> W [128, 32] bf16 per 128-lookup tile, PE transpose of 4
     tiles at once (bf16 matmul vs identity) -> W^T stationaries at row
     groups 0/32/64/96, 4 bf16 matmuls against B^T -> psum, ACT/DVE copy to
     a chunk-wide SBUF tile, one contiguous-per-partition HWDGE DMA per
     chunk to the output slice.
"""

import numpy as np

import concourse.bacc as bacc
import concourse.bass as bass
import concourse.mybir as mybir
import concourse.tile as tile
from concourse import bass_utils
from concourse.ap import AP
from concourse.masks import make_identity

# Problem constants (hardcoded per the harness contract).
VOC = (100, 100, 50)  # a, b, c
EMB = (8, 16)  # d, e'
RANK = 32
E = EMB[0] * EMB[1]  # 128
N_CORES = 8
X_SHAPE = (1024, 200)
N_TOTAL = X_SHAPE[0] * X_SHAPE[1]  # 204800
P = 128

U01_ROWS = VOC[0] * VOC[1]  # 10000
ROW_PAD = 64  # table row = 64 f32 = 256 B (dma_gather elem_size constraint)


class Cfg:
    def __init__(self, n_core, chunks):
        assert n_core % P == 0
        self.n_core = n_core
        self.n_tiles = n_core // P
        self.chunks = list(chunks)  # tiles per chunk; each % 4 == 0
        assert sum(self.chunks) == self.n_tiles
        assert all(c % 4 == 0 for c in self.chunks)
        self.max_chunk = max(self.chunks)
        self.idx_cols = n_core // 16  # wrapped idx layout [16 -> 128, idx_cols]


# 25600 lookups; small chunks bound the pipeline-fill latency and keep the
# gather/compute pipeline fine-grained across the 4 SWDGE queues.
FULL_CFG = Cfg(N_TOTAL // N_CORES, [12] * 16 + [8])

F32 = mybir.dt.float32
BF16 = mybir.dt.bfloat16
I16 = mybir.dt.int16

N_QUEUES = 4


def dma_gather_narrow(eng, out_ap, in_ap, idxs_ap, num_idxs, elem_size, elem_step,
                      queue_num=0):
    """dma_gather for the HBM-source non-transpose case with elem_size_bytes
    not necessarily a multiple of 256 (the Q7 ucode loops over bytes; only the
    row STRIDE must stay 256B-aligned). Mirrors bass.dma_gather's lowering.
    HW-validated: gathering 128B rows from a 256B-strided table is exact."""
    nc = eng.bass
    stride_bytes = elem_step * mybir.dt.size(in_ap.dtype)
    assert stride_bytes % 256 == 0 and stride_bytes // 256 < 256
    assert idxs_ap.dtype == I16
    _in_ap = eng.lower_ap_dma(in_ap, for_custom_bir_dma=True)
    _idxs_ap = eng.lower_ap(idxs_ap)
    _out_ap = eng.lower_ap(out_ap)
    return eng.add_instruction(
        mybir.InstDMAGatherAnt(
            name=nc.get_next_instruction_name(),
            ins=[*_in_ap, _idxs_ap, eng.lower_val_access(eng.to_reg(num_idxs))],
            outs=[_out_ap],
            transpose=False,
            num_idxs=num_idxs,
            elem_size=elem_size,
            stride_bytes_256=stride_bytes // 256,
            gen_mode=0,
            single_packet=False,
            queue_num=queue_num,
            sbuf_tokens_per_rank=0,
            sbuf_free_dim_per_rank=0,
            sbuf_free_dim_pad_per_rank=0,
            sbuf_byte_offset=0,
        )
    )


def build_program(cfg: Cfg, mode: str = "full"):
    """Build the SPMD single-core program; per-core differences are inputs.

    mode: "full" | "nogather" (memset the gather buffers instead of SWDGE
    gathers; for HW bisection only)."""
    nc = bacc.Bacc("TRN2", target_bir_lowering=False, debug=False,
                   num_swdge_queues=N_QUEUES)

    # ---- DRAM I/O ----
    jidx_d = nc.dram_tensor("jidx", [P, cfg.idx_cols], I16, kind="ExternalInput")
    cidx_d = nc.dram_tensor("cidx", [P, cfg.idx_cols], I16, kind="ExternalInput")
    u0_d = nc.dram_tensor("u0", [VOC[0], RANK], F32, kind="ExternalInput")
    u1rep_d = nc.dram_tensor("u1rep", [VOC[0], VOC[1] * RANK], F32, kind="ExternalInput")
    u2_d = nc.dram_tensor("u2", [VOC[2], RANK], F32, kind="ExternalInput")
    v0t4_d = nc.dram_tensor("v0t4", [P, EMB[0]], F32, kind="ExternalInput")
    v1t4_d = nc.dram_tensor("v1t4", [P, EMB[1]], F32, kind="ExternalInput")
    out_d = nc.dram_tensor("out", [cfg.n_core, E], F32, kind="ExternalOutput")
    # DRAM rows ordered so partition p's tiles are contiguous: row p*T + t.
    out_view = out_d.ap().rearrange("(p t) e -> p t e", p=P)

    with tile.TileContext(nc) as tc:
        const = tc.alloc_tile_pool(name="const", bufs=1)
        dram = tc.alloc_tile_pool(name="dram", bufs=1, space="DRAM")

        # ---------- one-time setup ----------
        # U01 table build heads the critical chain that gates the first real
        # gather: u1rep DMA -> DVE mul -> table DMA -> gather.  The mul and
        # the table DMA are split into four b-waves so they pipeline.
        # u01s[a, b, r] = U0[a, r] * U1[b, r]
        u0s = const.tile([VOC[0], RANK], F32)
        u1s = const.tile([VOC[0], VOC[1] * RANK], F32)
        nc.sync.dma_start(u0s[:], u0_d.ap())
        nc.sync.dma_start(u1s[:], u1rep_d.ap())
        # The build tile carries the full 64-f32 padded rows (zero pad in
        # cols 32:63) so each wave's DMA is one contiguous run per partition
        # (~100 descriptors) instead of a 128B descriptor per table row
        # (10000 descriptors ~= 40us of startup stall on the desc-rate
        # limited SDMA engines, which gated the first gather).
        u01s = const.tile([VOC[0], VOC[1] * ROW_PAD], F32)
        nc.gpsimd.memset(
            u01s[:].rearrange("p (b r) -> p b r", r=ROW_PAD)[:, :, RANK:ROW_PAD],
            0.0,
        )
        u01_tab = dram.tile([U01_ROWS, ROW_PAD], F32)
        NB = 4
        BH = VOC[1] // NB
        for h in range(NB):
            bs = slice(h * BH, (h + 1) * BH)
            nc.vector.tensor_tensor(
                out=u01s[:].rearrange("p (b r) -> p b r", r=ROW_PAD)[:, bs, 0:RANK],
                in0=u0s[:][:, None, :].to_broadcast([VOC[0], BH, RANK]),
                in1=u1s[:].rearrange("p (b r) -> p b r", r=RANK)[:, bs, :],
                op=mybir.AluOpType.mult,
            )
            nc.sync.dma_start(
                u01_tab[:].rearrange("(a b) r -> a b r", a=VOC[0])[:, bs, :],
                u01s[:].rearrange("p (b r) -> p b r", r=ROW_PAD)[:, bs, :],
            )

        # U2 table (tiny, separate tensor) so its gathers' descriptor
        # generation is not gated on the U01 table build.
        u2s = const.tile([VOC[2], RANK], F32)
        nc.scalar.dma_start(u2s[:], u2_d.ap())
        u2_tab = dram.tile([VOC[2], ROW_PAD], F32)
        nc.scalar.dma_start(u2_tab[:][:, 0:RANK], u2s[:])

        # idx tiles (gate only the Pool-engine descriptor generation)
        jidx = const.tile([P, cfg.idx_cols], I16)
        cidx = const.tile([P, cfg.idx_cols], I16)
        nc.scalar.dma_start(jidx[:], jidx_d.ap())
        nc.scalar.dma_start(cidx[:], cidx_d.ap())

        ident = const.tile([P, P], BF16)
        make_identity(nc, ident[:])

        # B^T (bf16) replicated at the 4 partition blocks:
        # bt[32g + r, d*16+e'] = V0[d, r] * V1[e', r]
        v0s = const.tile([P, EMB[0]], F32)
        v1s = const.tile([P, EMB[1]], F32)
        nc.scalar.dma_start(v0s[:], v0t4_d.ap())
        nc.scalar.dma_start(v1s[:], v1t4_d.ap())
        bt = const.tile([P, E], BF16)
        nc.vector.tensor_tensor(
            out=bt[:].rearrange("p (d e) -> p d e", e=EMB[1]),
            in0=v0s[:][:, :, None].to_broadcast([P, EMB[0], EMB[1]]),
            in1=v1s[:][:, None, :].to_broadcast([P, EMB[0], EMB[1]]),
            op=mybir.AluOpType.mult,
        )

        # ---------- main pipeline ----------
        g1p = tc.alloc_tile_pool(name="g1", bufs=4)
        g2p = tc.alloc_tile_pool(name="g2", bufs=4)
        wp = tc.alloc_tile_pool(name="w", bufs=2)
        wtpp = tc.alloc_tile_pool(name="wtp", bufs=2, space="PSUM")
        wtsp = tc.alloc_tile_pool(name="wts", bufs=4)
        opp = tc.alloc_tile_pool(name="op", bufs=6, space="PSUM")
        osp = tc.alloc_tile_pool(name="os", bufs=2)

        tile0 = 0
        for ch, ctiles in enumerate(cfg.chunks):
            cidx0 = tile0 * P // 16
            icols = ctiles * P // 16
            g1 = g1p.tile([P, ctiles, RANK], F32, tag="g1")
            g2 = g2p.tile([P, ctiles, RANK], F32, tag="g2")
            if mode == "nogather":
                nc.gpsimd.memset(g1[:], 1.0)
                nc.gpsimd.memset(g2[:], 1.0)
            else:
                # Round-robin both gathers of a chunk onto different SWDGE
                # queues; each queue has a dedicated Q7 cpu pair, so 4 queues
                # give 4x the descriptor generation rate.
                dma_gather_narrow(
                    nc.gpsimd, g2[:], u2_tab[:][:, 0:RANK],
                    cidx[:][:, cidx0 : cidx0 + icols],
                    ctiles * P, RANK, ROW_PAD,
                    queue_num=(2 * ch) % N_QUEUES,
                )
                dma_gather_narrow(
                    nc.gpsimd, g1[:], u01_tab[:][:, 0:RANK],
                    jidx[:][:, cidx0 : cidx0 + icols],
                    ctiles * P, RANK, ROW_PAD,
                    queue_num=(2 * ch + 1) % N_QUEUES,
                )
            w = wp.tile([P, cfg.max_chunk * RANK], BF16, tag="w")
            out_sb = osp.tile([P, cfg.max_chunk * E], F32, tag="os")
            for pk in range(ctiles // 4):
                # per-pack W multiply: finer grain lets the first transpose
                # start as soon as the gathers land
                nc.vector.tensor_tensor(
                    out=w[:].rearrange("p (t r) -> p t r", r=RANK)[
                        :, pk * 4 : (pk + 1) * 4, :
                    ],
                    in0=g1[:][:, pk * 4 : (pk + 1) * 4, :],
                    in1=g2[:][:, pk * 4 : (pk + 1) * 4, :],
                    op=mybir.AluOpType.mult,
                )
                # W^T via plain matmul against identity (fp32 is_transpose
                # crashes the exec unit on this stack; W.T @ I is exact).
                wt_ps = wtpp.tile([P, P], F32)
                nc.tensor.matmul(
                    out=wt_ps[:],
                    lhsT=w[:][:, pk * P : (pk + 1) * P],
                    rhs=ident[:],
                    start=True,
                    stop=True,
                )
                wt = wtsp.tile([P, P], BF16)
                nc.vector.tensor_copy(wt[:], wt_ps[:])
                for g in range(4):
                    out_ps = opp.tile([P, E], F32, tag="ops")
                    nc.tensor.matmul(
                        out=out_ps[:],
                        lhsT=wt[:][g * RANK : (g + 1) * RANK, :],
                        rhs=bt[:][g * RANK : (g + 1) * RANK, :],
                        start=True,
                        stop=True,
                        tile_position=(g * RANK, 0),
                    )
                    dst = out_sb[:][:, (pk * 4 + g) * E : (pk * 4 + g + 1) * E]
                    if g % 2 == 0:
                        nc.scalar.copy(dst, out_ps[:])
                    else:
                        nc.vector.tensor_copy(dst, out_ps[:])
            # One DMA per chunk: partition p's ctiles rows are contiguous in
            # DRAM (rows p*T + tile0 .. + ctiles), so each partition is one
            # multi-KB descriptor run instead of per-row 512B descriptors.
            nc.sync.dma_start(
                out_view[:, tile0 : tile0 + ctiles, :],
                out_sb[:][:, 0 : ctiles * E].rearrange("p (t e) -> p t e", e=E),
            )
            tile0 += ctiles

        for pool in (osp, opp, wtsp, wtpp, wp, g2p, g1p, dram, const):
            pool.release()

    nc.compile()
    return nc


def wrap_idx(v: np.ndarray) -> np.ndarray:
    """Host-side routing prep: dma_gather wants index i at [i % 16, i // 16],
    replicated down all 128 partitions (8 Q7 cores x 16 partitions)."""
    w = v.astype(np.int16).reshape(-1, 16).T  # [16, cols]
    return np.ascontiguousarray(np.tile(w, (8, 1)))  # [128, cols]


_CACHE: dict = {}


def _get_program(cfg: Cfg):
    key = (cfg.n_core, tuple(cfg.chunks))
    if key not in _CACHE:
        _CACHE[key] = build_program(cfg)
    return _CACHE[key]


def make_in_maps(x, U0, U1, U2, V0, V1, cfg: Cfg, n_cores: int):
    xf = np.asarray(x).reshape(-1).astype(np.int64)
    j = (xf // VOC[2]).astype(np.int16)  # [0, 10000)
    c = (xf % VOC[2]).astype(np.int16)  # [0, 50)

    u0 = np.ascontiguousarray(np.asarray(U0, dtype=np.float32))
    u1rep = np.ascontiguousarray(
        np.broadcast_to(
            np.asarray(U1, dtype=np.float32).reshape(1, VOC[1] * RANK),
            (VOC[0], VOC[1] * RANK),
        )
    )
    u2 = np.ascontiguousarray(np.asarray(U2, dtype=np.float32))
    v0t4 = np.ascontiguousarray(np.tile(np.asarray(V0, dtype=np.float32).T, (4, 1)))
    v1t4 = np.ascontiguousarray(np.tile(np.asarray(V1, dtype=np.float32).T, (4, 1)))

    T = cfg.n_tiles
    in_maps = []
    for k in range(n_cores):
        sl = slice(k * cfg.n_core, (k + 1) * cfg.n_core)
        # Gather-row g serves (slot t=g//128, partition p=g%128) = lookup
        # p*T + t, so permute the per-core index vector accordingly.
        jg = j[sl].reshape(P, T).T.reshape(-1)
        cg = c[sl].reshape(P, T).T.reshape(-1)
        in_maps.append(
            {
                "jidx": wrap_idx(jg),
                "cidx": wrap_idx(cg),
                "u0": u0,
                "u1rep": u1rep,
                "u2": u2,
                "v0t4": v0t4,
                "v1t4": v1t4,
            }
        )
    return in_maps


def kernel(x, U0, U1, U2, V0, V1, _trace=False):
    cfg = FULL_CFG
    nc = _get_program(cfg)
    in_maps = make_in_maps(x, U0, U1, U2, V0, V1, cfg, N_CORES)
    res = bass_utils.run_bass_kernel_spmd(
        nc, in_maps, core_ids=list(range(N_CORES)), trace=_trace
    )
    outs = []
    for k in range(N_CORES):
        # Device rows are ordered (p, t) = lookup p*T + t: already the
        # natural per-core order.
        outs.append(res.results[k]["out"])
    out = np.concatenate(outs, axis=0)
    out = out.reshape(*np.asarray(x).shape, E).astype(np.float32)
    if _trace:
        kernel._last_result = res
    return out
